# revision 4
# baseline (speedup 1.0000x reference)
"""Trainium2 Bass kernel for nn_Disentangler (gnn_message_passing).

Math (per timestamp t, derived from the reference):
  - encode LayerNorm over D of x rows; only rows at node_pos are used.
  - scatter to nodes by node_ids, adaptive-avg-pool D->C, segment-mean over
    L=8 groups of 4096 nodes  ==>  comp[l] = A1 @ (sum over selected rows p
    with node_ids//4096==l of LN(x_p)) / 4096.
    LN(x_p) = (x_p - m_p) * r_p * g_enc + b_enc with per-row mean m_p and
    r_p = 1/sqrt(var_p + eps).  So the bucket sums only need
    W_l = sum r_p x_p (a one-hot-weighted matmul) plus S_l = sum r_p m_p =
    reduce_sum(W_l)/D, and counts cnt_l.
  - LN over the L*C=128 comp values (g_fin/b_fin), then per-group LN over
    C=16 (g_dec/b_dec), then upsample C->D by repeat-8.
  - out rows within a group are all identical -> write each group's 4096
    identical rows via a stride-0 broadcast DMA from an [L, 2048] fp16 tile.

Implementation notes (v2, memory-roofline focused):
  - host pre-gathers the P=8192 selected rows per timestamp (node_pos) and
    ships them fp16: read traffic 4.2 MB/core instead of 16.8 MB full-x f32.
  - bucket one-hot is built on-chip from a [128, 64] bucket-id tensor via
    gpsimd is_equal against an iota, then scaled by the per-row r.
  - output y is fp16 on device (host upcasts during unshard): write traffic
    16.8 MB/core instead of 33.6.  fp16 error ~5e-4 << 2e-2 tolerance.
  - all x-chunk loads are issued up front on the sync queue, before the
    y-write DMAs, so loads never queue behind compute-blocked writes.
  - y write is ONE dma_start per timestamp: [8, 256, 2048] AP, stride-0
    source re-read, 4KB packets spread across all 16 DMA engines.

Sharding: data-parallel over T=16 timestamps across 8 cores (2 per core).
"""

import numpy as np

import concourse.bass as bass
import concourse.bacc as bacc
import concourse.tile as tile
from concourse import mybir
from concourse.bass_utils import run_bass_kernel_spmd

F32 = mybir.dt.float32
F16 = mybir.dt.float16
AF = mybir.ActivationFunctionType
ALU = mybir.AluOpType
AX = mybir.AxisListType

T, TOK, D, N, L, C = 16, 16384, 128, 32768, 8, 16
P = 8192                    # selected rows per timestamp
NCORES = 8
TLOC = T // NCORES          # timestamps per core
NT = P // 128               # 64 row-tiles per timestamp
CH = 4                      # x chunks per timestamp
JPC = NT // CH              # 16 tiles per chunk
GRP = N // L                # 4096 nodes per group
EPS = 1e-5
POOL_SCALE = 1.0 / ((D // C) * GRP)   # A1 avg (1/8) * segment mean (1/4096)
RW = 2048                   # replicated row-image width (16 copies of D)

_CACHE = {}


def _build():
    nc = bacc.Bacc("TRN2", debug=False)
    xs = nc.dram_tensor("xs", [TLOC, P, D], F16, kind="ExternalInput")
    bid = nc.dram_tensor("bid", [TLOC, 128, NT], F32, kind="ExternalInput")
    cnt = nc.dram_tensor("cnt", [TLOC, L, 1], F32, kind="ExternalInput")
    iot = nc.dram_tensor("iot", [128, L], F32, kind="ExternalInput")
    geb = nc.dram_tensor("geb", [L, D], F32, kind="ExternalInput")
    beb = nc.dram_tensor("beb", [L, D], F32, kind="ExternalInput")
    gft = nc.dram_tensor("gft", [L, C], F32, kind="ExternalInput")
    bft = nc.dram_tensor("bft", [L, C], F32, kind="ExternalInput")
    gdt = nc.dram_tensor("gdt", [L, C], F32, kind="ExternalInput")
    bdt = nc.dram_tensor("bdt", [L, C], F32, kind="ExternalInput")
    ones8 = nc.dram_tensor("ones8", [L, 1], F32, kind="ExternalInput")
    ones18 = nc.dram_tensor("ones18", [1, L], F32, kind="ExternalInput")
    y = nc.dram_tensor("y", [TLOC, N, D], F16, kind="ExternalOutput")

    with tile.TileContext(nc) as tc:
        with (
            tc.tile_pool(name="xp", bufs=1) as xp,
            tc.tile_pool(name="bidp", bufs=1) as bidp,
            tc.tile_pool(name="sqp", bufs=2) as sqp,
            tc.tile_pool(name="selp", bufs=2) as selp,
            tc.tile_pool(name="stat", bufs=4) as stat,
            tc.tile_pool(name="mid", bufs=2) as mid,
            tc.tile_pool(name="rep", bufs=2) as repp,
            tc.tile_pool(name="const", bufs=1) as cst,
            tc.tile_pool(name="psw", bufs=2, space="PSUM") as psw,
            tc.tile_pool(name="pst", bufs=2, space="PSUM") as pst,
        ):
            # ---- x chunk loads: issue ALL of them first on the sync queue
            # (no input deps -> rings start immediately; the y writes queued
            # later on the same engine can never head-of-line block these).
            xch = []
            for t in range(TLOC):
                xr = xs[t].rearrange("(p j) d -> p j d", p=128)
                for c in range(CH):
                    xc = xp.tile([128, JPC, D], F16, tag=f"x{t}c{c}")
                    nc.sync.dma_start(out=xc[:], in_=xr[:, c * JPC:(c + 1) * JPC, :])
                    xch.append(xc)

            # ---- constants + per-t metadata (scalar queue) ----
            bid_s = []
            cnt_s = []
            for t in range(TLOC):
                b = bidp.tile([128, NT], F32, tag=f"bid{t}")
                nc.scalar.dma_start(out=b[:], in_=bid[t])
                bid_s.append(b)
                cs = mid.tile([L, 1], F32, tag=f"cnt{t}")
                nc.scalar.dma_start(out=cs[:], in_=cnt[t])
                cnt_s.append(cs)
            iot_s = cst.tile([128, L], F32); nc.scalar.dma_start(out=iot_s[:], in_=iot[:])
            geb_s = cst.tile([L, D], F32); nc.scalar.dma_start(out=geb_s[:], in_=geb[:])
            beb_s = cst.tile([L, D], F32); nc.scalar.dma_start(out=beb_s[:], in_=beb[:])
            gft_s = cst.tile([L, C], F32); nc.scalar.dma_start(out=gft_s[:], in_=gft[:])
            bft_s = cst.tile([L, C], F32); nc.scalar.dma_start(out=bft_s[:], in_=bft[:])
            gdt_s = cst.tile([L, C], F32); nc.scalar.dma_start(out=gdt_s[:], in_=gdt[:])
            bdt_s = cst.tile([L, C], F32); nc.scalar.dma_start(out=bdt_s[:], in_=bdt[:])
            on8_s = cst.tile([L, 1], F32); nc.scalar.dma_start(out=on8_s[:], in_=ones8[:])
            on18_s = cst.tile([1, L], F32); nc.scalar.dma_start(out=on18_s[:], in_=ones18[:])
            eps_s = cst.tile([128, 1], F32); nc.vector.memset(eps_s[:], EPS)
            eps2_s = cst.tile([1, 1], F32); nc.vector.memset(eps2_s[:], EPS / (POOL_SCALE * POOL_SCALE))

            for t in range(TLOC):
                ps_w = psw.tile([L, D], F32)   # accumulates W over all tiles
                for c in range(CH):
                    xc = xch[t * CH + c]
                    sums = stat.tile([128, JPC], F32, tag="sums")
                    nc.vector.reduce_sum(out=sums[:], in_=xc[:], axis=AX.X)
                    xsq = sqp.tile([128, JPC * D], F16)
                    nc.scalar.activation(out=xsq[:],
                                         in_=xc[:].rearrange("p j d -> p (j d)"),
                                         func=AF.Square)
                    sumsq = stat.tile([128, JPC], F32, tag="sumsq")
                    nc.vector.reduce_sum(out=sumsq[:],
                                         in_=xsq[:].rearrange("p (j d) -> p j d", d=D),
                                         axis=AX.X)
                    s2 = stat.tile([128, JPC], F32, tag="s2")
                    nc.gpsimd.tensor_mul(out=s2[:], in0=sums[:], in1=sums[:])
                    nc.gpsimd.tensor_scalar(out=s2[:], in0=s2[:], scalar1=1.0 / D,
                                            scalar2=None, op0=ALU.mult)
                    nc.gpsimd.tensor_tensor(out=s2[:], in0=sumsq[:], in1=s2[:],
                                            op=ALU.subtract)
                    r16 = stat.tile([128, JPC], F16, tag="r")
                    nc.scalar.activation(out=r16[:], in_=s2[:], func=AF.Abs_reciprocal_sqrt,
                                         bias=eps_s[:], scale=1.0 / D)
                    sel = selp.tile([128, JPC, L], F16)
                    nc.vector.tensor_tensor(
                        out=sel[:],
                        in0=bid_s[t][:, c * JPC:(c + 1) * JPC].rearrange(
                            "p (j o) -> p j o", o=1).to_broadcast([128, JPC, L]),
                        in1=iot_s[:].rearrange("p (o l) -> p o l", o=1).to_broadcast(
                            [128, JPC, L]),
                        op=ALU.is_equal)
                    nc.vector.tensor_tensor(
                        out=sel[:], in0=sel[:],
                        in1=r16[:].rearrange("p (j o) -> p j o", o=1).to_broadcast(
                            [128, JPC, L]),
                        op=ALU.mult)
                    for jj in range(JPC):
                        j = c * JPC + jj
                        nc.tensor.matmul(ps_w[:], lhsT=sel[:, jj, :], rhs=xc[:, jj, :],
                                         start=(j == 0), stop=(j == NT - 1))

                # ---- per-timestamp tail (all tiny, f32) ----
                S = mid.tile([L, 1], F32, tag="S")
                nc.vector.reduce_sum(out=S[:], in_=ps_w[:], axis=AX.X)
                nc.vector.tensor_scalar(out=S[:], in0=S[:], scalar1=1.0 / D,
                                        scalar2=None, op0=ALU.mult)
                t1 = mid.tile([L, D], F32, tag="t1")
                nc.vector.tensor_scalar(out=t1[:], in0=ps_w[:], scalar1=S[:],
                                        scalar2=None, op0=ALU.subtract)
                nc.vector.tensor_mul(out=t1[:], in0=t1[:], in1=geb_s[:])
                cb = mid.tile([L, D], F32, tag="cb")
                nc.vector.tensor_scalar_mul(out=cb[:], in0=beb_s[:], scalar1=cnt_s[t][:])
                nc.vector.tensor_add(out=t1[:], in0=t1[:], in1=cb[:])

                cp = mid.tile([L, C], F32, tag="cp")
                nc.vector.reduce_sum(out=cp[:], in_=t1[:].rearrange("l (c g) -> l c g", g=D // C),
                                     axis=AX.X)

                # LN over all L*C values: stats via ones-matmul partition sum
                sq = mid.tile([L, C], F32, tag="sq")
                nc.vector.tensor_mul(out=sq[:], in0=cp[:], in1=cp[:])
                ps2 = pst.tile([1, 2 * C], F32, tag="tail")
                nc.tensor.matmul(ps2[:, :C], lhsT=on8_s[:], rhs=cp[:], start=True, stop=True)
                nc.tensor.matmul(ps2[:, C:], lhsT=on8_s[:], rhs=sq[:], start=True, stop=True)
                su = mid.tile([1, 2], F32, tag="su")
                nc.vector.reduce_sum(out=su[:], in_=ps2[:].rearrange("p (a c) -> p a c", a=2),
                                     axis=AX.X)
                mst = mid.tile([1, 2], F32, tag="mst")
                nc.vector.tensor_scalar(out=mst[:], in0=su[:], scalar1=1.0 / (L * C),
                                        scalar2=None, op0=ALU.mult)  # [mean, meansq]
                msq = mid.tile([1, 1], F32, tag="msq")
                nc.vector.tensor_mul(out=msq[:], in0=mst[:, 0:1], in1=mst[:, 0:1])
                var = mid.tile([1, 1], F32, tag="var")
                nc.vector.tensor_tensor(out=var[:], in0=mst[:, 1:2], in1=msq[:],
                                        op=ALU.subtract)
                nc.scalar.activation(out=mst[:, 1:2], in_=var[:], func=AF.Abs_reciprocal_sqrt,
                                     bias=eps2_s[:1, :], scale=1.0)
                psb2 = pst.tile([L, 2], F32, tag="tail")
                nc.tensor.matmul(psb2[:], lhsT=on18_s[:], rhs=mst[:], start=True, stop=True)
                bsb = mid.tile([L, 2], F32, tag="bsb")
                nc.vector.tensor_copy(out=bsb[:], in_=psb2[:])

                cl = mid.tile([L, C], F32, tag="cl")
                nc.vector.tensor_scalar(out=cl[:], in0=cp[:], scalar1=bsb[:, 0:1],
                                        scalar2=bsb[:, 1:2],
                                        op0=ALU.subtract, op1=ALU.mult)
                nc.vector.tensor_mul(out=cl[:], in0=cl[:], in1=gft_s[:])
                nc.vector.tensor_add(out=cl[:], in0=cl[:], in1=bft_s[:])

                # decode LN over C per group
                st2 = mid.tile([L, nc.vector.BN_STATS_DIM], F32, tag="st2")
                nc.vector.bn_stats(out=st2[:], in_=cl[:])
                mv2 = mid.tile([L, 2], F32, tag="mv2")
                nc.vector.bn_aggr(out=mv2[:], in_=st2[:])
                r2 = mid.tile([L, 1], F32, tag="r2")
                nc.scalar.activation(out=r2[:], in_=mv2[:, 1:2], func=AF.Abs_reciprocal_sqrt,
                                     bias=eps_s[:L, :], scale=1.0)
                dn = mid.tile([L, C], F32, tag="dn")
                nc.vector.tensor_scalar(out=dn[:], in0=cl[:], scalar1=mv2[:, 0:1],
                                        scalar2=r2[:],
                                        op0=ALU.subtract, op1=ALU.mult)
                nc.vector.tensor_mul(out=dn[:], in0=dn[:], in1=gdt_s[:])
                nc.vector.tensor_add(out=dn[:], in0=dn[:], in1=bdt_s[:])

                # upsample C -> D (repeat 8), tiled 16x to width RW, fp16
                rw = repp.tile([L, RW], F16)
                nc.vector.tensor_copy(
                    out=rw[:].rearrange("l (r c k) -> l r c k", r=RW // D, k=D // C),
                    in_=dn[:].rearrange("l (o c u) -> l o c u", o=1, u=1).to_broadcast(
                        [L, RW // D, C, D // C]))

                # one write DMA per timestamp: each group row-image is re-read
                # (stride 0) 256x and written as 4KB packets; the runtime
                # spreads packets across all DMA engines.
                out_ap = y[t].rearrange("(l a f) d -> l a (f d)", l=L, a=GRP * D // RW)
                in_ap = rw[:].rearrange("l (o f) -> l o f", o=1).to_broadcast(
                    [L, GRP * D // RW, RW])
                nc.sync.dma_start(out=out_ap, in_=in_ap)

    nc.compile()
    return nc


def _get_nc():
    if "nc" not in _CACHE:
        _CACHE["nc"] = _build()
    return _CACHE["nc"]


def _host_prep(x, g_enc, b_enc, g_fin, b_fin, g_dec, b_dec, node_pos, node_ids):
    """Build per-core input maps: gather selected rows (fp16) + bucket ids."""
    consts = {
        "iot": np.tile(np.arange(L, dtype=np.float32), (128, 1)),
        "geb": np.tile(np.asarray(g_enc, np.float32), (L, 1)),
        "beb": np.tile(np.asarray(b_enc, np.float32), (L, 1)),
        "gft": np.asarray(g_fin, np.float32).reshape(L, C),
        "bft": np.asarray(b_fin, np.float32).reshape(L, C),
        "gdt": np.tile(np.asarray(g_dec, np.float32), (L, 1)),
        "bdt": np.tile(np.asarray(b_dec, np.float32), (L, 1)),
        "ones8": np.ones((L, 1), np.float32),
        "ones18": np.ones((1, L), np.float32),
    }
    x = np.asarray(x)
    node_pos = np.asarray(node_pos)
    buckets = (np.asarray(node_ids) // GRP).astype(np.int64)          # [T, P]
    in_maps = []
    for core in range(NCORES):
        xs = np.empty((TLOC, P, D), np.float16)
        bid = np.empty((TLOC, 128, NT), np.float32)
        cnt = np.empty((TLOC, L), np.float32)
        for i, t in enumerate(range(core * TLOC, (core + 1) * TLOC)):
            xs[i] = x[t][node_pos[t]]
            bid[i] = buckets[t].astype(np.float32).reshape(128, NT)
            cnt[i] = np.bincount(buckets[t], minlength=L).astype(np.float32)
        in_maps.append({
            "xs": xs,
            "bid": bid,
            "cnt": cnt.reshape(TLOC, L, 1),
            **consts,
        })
    return in_maps


def kernel(**inputs):
    in_maps = _host_prep(
        inputs["x"], inputs["g_enc"], inputs["b_enc"], inputs["g_fin"], inputs["b_fin"],
        inputs["g_dec"], inputs["b_dec"], inputs["node_pos"], inputs["node_ids"])
    nc = _get_nc()
    res = run_bass_kernel_spmd(nc, in_maps, core_ids=list(range(NCORES)))
    out = np.concatenate([r["y"] for r in res.results], axis=0)
    return out.astype(np.float32)


# revision 13
# speedup vs baseline: 3.4626x; 3.4626x over previous
"""Trainium2 Bass kernel for nn_Disentangler (gnn_message_passing).

Math (per timestamp t, derived from the reference):
  - encode LayerNorm over D of x rows; only rows at node_pos are used.
  - scatter to nodes by node_ids, adaptive-avg-pool D->C, segment-mean over
    L=8 groups of 4096 nodes  ==>  comp[l] = A1 @ (sum over selected rows p
    with node_ids//4096==l of LN(x_p)) / 4096.
    LN(x_p) = (x_p - m_p) * r_p * g_enc + b_enc with per-row mean m_p and
    r_p = 1/sqrt(var_p + eps).  So the bucket sums only need
    W_l = sum r_p x_p (a one-hot-weighted matmul) plus S_l = sum r_p m_p =
    reduce_sum(W_l)/D, and counts cnt_l.
  - LN over the L*C=128 comp values (g_fin/b_fin), then per-group LN over
    C=16 (g_dec/b_dec), then upsample C->D by repeat-8.
  - out rows within a group are all identical -> write each group's 4096
    identical rows via a stride-0 broadcast DMA from an [L, 2048] fp16 tile.

Implementation notes (v2, memory-roofline focused):
  - host pre-gathers the P=8192 selected rows per timestamp (node_pos) and
    ships them fp16: read traffic 4.2 MB/core instead of 16.8 MB full-x f32.
  - bucket one-hot is built on-chip from a [128, 64] bucket-id tensor via
    gpsimd is_equal against an iota, then scaled by the per-row r.
  - output y is fp16 on device (host upcasts during unshard): write traffic
    16.8 MB/core instead of 33.6.  fp16 error ~5e-4 << 2e-2 tolerance.
  - all x-chunk loads are issued up front on the sync queue, before the
    y-write DMAs, so loads never queue behind compute-blocked writes.
  - y write is ONE dma_start per timestamp: [8, 256, 2048] AP, stride-0
    source re-read, 4KB packets spread across all 16 DMA engines.

Sharding: data-parallel over T=16 timestamps across 8 cores (2 per core).
"""

import numpy as np

import concourse.bass as bass
import concourse.bacc as bacc
import concourse.tile as tile
from concourse import mybir
from concourse.bass_utils import run_bass_kernel_spmd

F32 = mybir.dt.float32
F16 = mybir.dt.float16
AF = mybir.ActivationFunctionType
ALU = mybir.AluOpType
AX = mybir.AxisListType

T, TOK, D, N, L, C = 16, 16384, 128, 32768, 8, 16
P = 8192                    # selected rows per timestamp
NCORES = 8
TLOC = T // NCORES          # timestamps per core
NT = P // 128               # 64 row-tiles per timestamp
CH = 4                      # x chunks per timestamp
JPC = NT // CH              # 16 tiles per chunk
GRP = N // L                # 4096 nodes per group
EPS = 1e-5
POOL_SCALE = 1.0 / ((D // C) * GRP)   # A1 avg (1/8) * segment mean (1/4096)
RW = 2048                   # replicated row-image width (16 copies of D)

_CACHE = {}


def _build():
    nc = bacc.Bacc("TRN2", debug=False)
    xs = nc.dram_tensor("xs", [TLOC, P, D], F16, kind="ExternalInput")
    bid = nc.dram_tensor("bid", [TLOC, 128, NT], F32, kind="ExternalInput")
    cnt = nc.dram_tensor("cnt", [TLOC, L, 1], F32, kind="ExternalInput")
    iot = nc.dram_tensor("iot", [128, L], F32, kind="ExternalInput")
    geb = nc.dram_tensor("geb", [L, D], F32, kind="ExternalInput")
    beb = nc.dram_tensor("beb", [L, D], F32, kind="ExternalInput")
    gft = nc.dram_tensor("gft", [L, C], F32, kind="ExternalInput")
    bft = nc.dram_tensor("bft", [L, C], F32, kind="ExternalInput")
    gdt = nc.dram_tensor("gdt", [L, C], F32, kind="ExternalInput")
    bdt = nc.dram_tensor("bdt", [L, C], F32, kind="ExternalInput")
    ones8 = nc.dram_tensor("ones8", [L, 1], F32, kind="ExternalInput")
    ones18 = nc.dram_tensor("ones18", [1, L], F32, kind="ExternalInput")
    bsel = nc.dram_tensor("bsel", [L, L * 128], F16, kind="ExternalInput")
    y = nc.dram_tensor("y", [TLOC, N, D], F16, kind="ExternalOutput")

    with tile.TileContext(nc) as tc:
        with (
            tc.tile_pool(name="xp", bufs=1) as xp,
            tc.tile_pool(name="bidp", bufs=1) as bidp,
            tc.tile_pool(name="sqp", bufs=2) as sqp,
            tc.tile_pool(name="selp", bufs=2) as selp,
            tc.tile_pool(name="stat", bufs=4) as stat,
            tc.tile_pool(name="mid", bufs=2) as mid,
            tc.tile_pool(name="rep", bufs=2) as repp,
            tc.tile_pool(name="repg", bufs=4) as repg,
            tc.tile_pool(name="const", bufs=1) as cst,
            tc.tile_pool(name="psw", bufs=2, space="PSUM") as psw,
            tc.tile_pool(name="pst", bufs=2, space="PSUM") as pst,
            tc.tile_pool(name="psb", bufs=2, space="PSUM") as psb,
        ):
            # ---- x chunk loads: issue ALL of them first on the sync queue
            # (no input deps -> rings start immediately; the y writes queued
            # later on the same engine can never head-of-line block these).
            xch = []
            for t in range(TLOC):
                xr = xs[t].rearrange("(p j) d -> p j d", p=128)
                for c in range(CH):
                    xc = xp.tile([128, JPC, D], F16, tag=f"x{t}c{c}")
                    nc.sync.dma_start(out=xc[:], in_=xr[:, c * JPC:(c + 1) * JPC, :])
                    xch.append(xc)

            # ---- constants + per-t metadata (scalar queue) ----
            bid_s = []
            cnt_s = []
            for t in range(TLOC):
                b = bidp.tile([128, NT], F32, tag=f"bid{t}")
                nc.scalar.dma_start(out=b[:], in_=bid[t])
                bid_s.append(b)
                cs = mid.tile([L, 1], F32, tag=f"cnt{t}")
                nc.scalar.dma_start(out=cs[:], in_=cnt[t])
                cnt_s.append(cs)
            iot_s = cst.tile([128, L], F32); nc.scalar.dma_start(out=iot_s[:], in_=iot[:])
            geb_s = cst.tile([L, D], F32); nc.scalar.dma_start(out=geb_s[:], in_=geb[:])
            beb_s = cst.tile([L, D], F32); nc.scalar.dma_start(out=beb_s[:], in_=beb[:])
            gft_s = cst.tile([L, C], F32); nc.scalar.dma_start(out=gft_s[:], in_=gft[:])
            bft_s = cst.tile([L, C], F32); nc.scalar.dma_start(out=bft_s[:], in_=bft[:])
            gdt_s = cst.tile([L, C], F32); nc.scalar.dma_start(out=gdt_s[:], in_=gdt[:])
            bdt_s = cst.tile([L, C], F32); nc.scalar.dma_start(out=bdt_s[:], in_=bdt[:])
            on8_s = cst.tile([L, 1], F32); nc.scalar.dma_start(out=on8_s[:], in_=ones8[:])
            on18_s = cst.tile([1, L], F32); nc.scalar.dma_start(out=on18_s[:], in_=ones18[:])
            bsel_s = cst.tile([L, L * 128], F16); nc.scalar.dma_start(out=bsel_s[:], in_=bsel[:])
            eps_s = cst.tile([128, 1], F32); nc.vector.memset(eps_s[:], EPS)
            eps2_s = cst.tile([1, 1], F32); nc.vector.memset(eps2_s[:], EPS / (POOL_SCALE * POOL_SCALE))

            for t in range(TLOC):
                ps_w = psw.tile([L, D], F32)   # accumulates W over all tiles
                for c in range(CH):
                    xc = xch[t * CH + c]
                    sums = stat.tile([128, JPC], F32, tag="sums")
                    nc.vector.reduce_sum(out=sums[:], in_=xc[:], axis=AX.X)
                    xsq = sqp.tile([128, JPC * D], F16)
                    nc.scalar.activation(out=xsq[:],
                                         in_=xc[:].rearrange("p j d -> p (j d)"),
                                         func=AF.Square)
                    sumsq = stat.tile([128, JPC], F32, tag="sumsq")
                    nc.vector.reduce_sum(out=sumsq[:],
                                         in_=xsq[:].rearrange("p (j d) -> p j d", d=D),
                                         axis=AX.X)
                    s2 = stat.tile([128, JPC], F32, tag="s2")
                    nc.gpsimd.tensor_mul(out=s2[:], in0=sums[:], in1=sums[:])
                    nc.gpsimd.tensor_scalar(out=s2[:], in0=s2[:], scalar1=1.0 / D,
                                            scalar2=None, op0=ALU.mult)
                    nc.gpsimd.tensor_tensor(out=s2[:], in0=sumsq[:], in1=s2[:],
                                            op=ALU.subtract)
                    r16 = stat.tile([128, JPC], F16, tag="r")
                    nc.scalar.activation(out=r16[:], in_=s2[:], func=AF.Abs_reciprocal_sqrt,
                                         bias=eps_s[:], scale=1.0 / D)
                    sel = selp.tile([128, JPC, L], F16)
                    nc.vector.tensor_tensor(
                        out=sel[:],
                        in0=bid_s[t][:, c * JPC:(c + 1) * JPC].rearrange(
                            "p (j o) -> p j o", o=1).to_broadcast([128, JPC, L]),
                        in1=iot_s[:].rearrange("p (o l) -> p o l", o=1).to_broadcast(
                            [128, JPC, L]),
                        op=ALU.is_equal)
                    nc.vector.tensor_tensor(
                        out=sel[:], in0=sel[:],
                        in1=r16[:].rearrange("p (j o) -> p j o", o=1).to_broadcast(
                            [128, JPC, L]),
                        op=ALU.mult)
                    for jj in range(JPC):
                        j = c * JPC + jj
                        nc.tensor.matmul(ps_w[:], lhsT=sel[:, jj, :], rhs=xc[:, jj, :],
                                         start=(j == 0), stop=(j == NT - 1))

                # ---- per-timestamp tail (all tiny, f32) ----
                S = mid.tile([L, 1], F32, tag="S")
                nc.vector.reduce_sum(out=S[:], in_=ps_w[:], axis=AX.X)
                nc.vector.tensor_scalar(out=S[:], in0=S[:], scalar1=1.0 / D,
                                        scalar2=None, op0=ALU.mult)
                t1 = mid.tile([L, D], F32, tag="t1")
                nc.vector.tensor_scalar(out=t1[:], in0=ps_w[:], scalar1=S[:],
                                        scalar2=None, op0=ALU.subtract)
                nc.vector.tensor_mul(out=t1[:], in0=t1[:], in1=geb_s[:])
                cb = mid.tile([L, D], F32, tag="cb")
                nc.vector.tensor_scalar_mul(out=cb[:], in0=beb_s[:], scalar1=cnt_s[t][:])
                nc.vector.tensor_add(out=t1[:], in0=t1[:], in1=cb[:])

                cp = mid.tile([L, C], F32, tag="cp")
                nc.vector.reduce_sum(out=cp[:], in_=t1[:].rearrange("l (c g) -> l c g", g=D // C),
                                     axis=AX.X)

                # LN over all L*C values: stats via ones-matmul partition sum
                sq = mid.tile([L, C], F32, tag="sq")
                nc.vector.tensor_mul(out=sq[:], in0=cp[:], in1=cp[:])
                ps2 = pst.tile([1, 2 * C], F32, tag="tail")
                nc.tensor.matmul(ps2[:, :C], lhsT=on8_s[:], rhs=cp[:], start=True, stop=True)
                nc.tensor.matmul(ps2[:, C:], lhsT=on8_s[:], rhs=sq[:], start=True, stop=True)
                su = mid.tile([1, 2], F32, tag="su")
                nc.vector.reduce_sum(out=su[:], in_=ps2[:].rearrange("p (a c) -> p a c", a=2),
                                     axis=AX.X)
                mst = mid.tile([1, 2], F32, tag="mst")
                nc.vector.tensor_scalar(out=mst[:], in0=su[:], scalar1=1.0 / (L * C),
                                        scalar2=None, op0=ALU.mult)  # [mean, meansq]
                msq = mid.tile([1, 1], F32, tag="msq")
                nc.vector.tensor_mul(out=msq[:], in0=mst[:, 0:1], in1=mst[:, 0:1])
                var = mid.tile([1, 1], F32, tag="var")
                nc.vector.tensor_tensor(out=var[:], in0=mst[:, 1:2], in1=msq[:],
                                        op=ALU.subtract)
                nc.scalar.activation(out=mst[:, 1:2], in_=var[:], func=AF.Abs_reciprocal_sqrt,
                                     bias=eps2_s[:1, :], scale=1.0)
                psb2 = pst.tile([L, 2], F32, tag="tail")
                nc.tensor.matmul(psb2[:], lhsT=on18_s[:], rhs=mst[:], start=True, stop=True)
                bsb = mid.tile([L, 2], F32, tag="bsb")
                nc.vector.tensor_copy(out=bsb[:], in_=psb2[:])

                cl = mid.tile([L, C], F32, tag="cl")
                nc.vector.tensor_scalar(out=cl[:], in0=cp[:], scalar1=bsb[:, 0:1],
                                        scalar2=bsb[:, 1:2],
                                        op0=ALU.subtract, op1=ALU.mult)
                nc.vector.tensor_mul(out=cl[:], in0=cl[:], in1=gft_s[:])
                nc.vector.tensor_add(out=cl[:], in0=cl[:], in1=bft_s[:])

                # decode LN over C per group
                st2 = mid.tile([L, nc.vector.BN_STATS_DIM], F32, tag="st2")
                nc.vector.bn_stats(out=st2[:], in_=cl[:])
                mv2 = mid.tile([L, 2], F32, tag="mv2")
                nc.vector.bn_aggr(out=mv2[:], in_=st2[:])
                r2 = mid.tile([L, 1], F32, tag="r2")
                nc.scalar.activation(out=r2[:], in_=mv2[:, 1:2], func=AF.Abs_reciprocal_sqrt,
                                     bias=eps_s[:L, :], scale=1.0)
                dn = mid.tile([L, C], F32, tag="dn")
                nc.vector.tensor_scalar(out=dn[:], in0=cl[:], scalar1=mv2[:, 0:1],
                                        scalar2=r2[:],
                                        op0=ALU.subtract, op1=ALU.mult)
                nc.vector.tensor_mul(out=dn[:], in0=dn[:], in1=gdt_s[:])
                nc.vector.tensor_add(out=dn[:], in0=dn[:], in1=bdt_s[:])

                # upsample C -> D (repeat 8), fp16
                rw = repp.tile([L, D], F16)
                nc.vector.tensor_copy(
                    out=rw[:].rearrange("l (c k) -> l c k", k=D // C),
                    in_=dn[:].rearrange("l (c u) -> l c u", u=1).to_broadcast(
                        [L, C, D // C]))

                # broadcast each group's row-image to all 128 partitions with
                # a narrow PE ones-column matmul ([128, D] only), then widen
                # 16x during the PSUM->SBUF fp16 cast copy (stride-0 input),
                # then write the group's 4096 identical rows with a 2x
                # stride-0 re-read: 4KB packets from 128 partitions.
                nrep = GRP * D // (128 * RW)
                for gl in range(L):
                    pb = psb.tile([128, D], F32)
                    nc.tensor.matmul(pb[:],
                                     lhsT=bsel_s[:, gl * 128:(gl + 1) * 128],
                                     rhs=rw[:], start=True, stop=True)
                    rep = repg.tile([128, RW], F16)
                    rep_out = rep[:].rearrange("p (r d) -> p r d", d=D)
                    rep_in = pb[:].rearrange("p (o d) -> p o d", o=1).to_broadcast(
                        [128, RW // D, D])
                    if gl % 2 == 0:
                        nc.scalar.copy(out=rep_out, in_=rep_in)
                    else:
                        nc.vector.tensor_copy(out=rep_out, in_=rep_in)
                    out_ap = y[t, gl * GRP:(gl + 1) * GRP, :].rearrange(
                        "(p a f) d -> p a (f d)", p=128, a=nrep)
                    in_ap = rep[:].rearrange("p (o f) -> p o f", o=1).to_broadcast(
                        [128, nrep, RW])
                    nc.sync.dma_start(out=out_ap, in_=in_ap)

    nc.compile()
    return nc


def _get_nc():
    if "nc" not in _CACHE:
        _CACHE["nc"] = _build()
    return _CACHE["nc"]


def _host_prep(x, g_enc, b_enc, g_fin, b_fin, g_dec, b_dec, node_pos, node_ids):
    """Build per-core input maps: gather selected rows (fp16) + bucket ids."""
    consts = {
        "iot": np.tile(np.arange(L, dtype=np.float32), (128, 1)),
        "geb": np.tile(np.asarray(g_enc, np.float32), (L, 1)),
        "beb": np.tile(np.asarray(b_enc, np.float32), (L, 1)),
        "gft": np.asarray(g_fin, np.float32).reshape(L, C),
        "bft": np.asarray(b_fin, np.float32).reshape(L, C),
        "gdt": np.tile(np.asarray(g_dec, np.float32), (L, 1)),
        "bdt": np.tile(np.asarray(b_dec, np.float32), (L, 1)),
        "ones8": np.ones((L, 1), np.float32),
        "ones18": np.ones((1, L), np.float32),
    }
    bsel = np.zeros((L, L * 128), np.float16)
    for l in range(L):
        bsel[l, l * 128:(l + 1) * 128] = 1.0
    consts["bsel"] = bsel
    x = np.asarray(x)
    node_pos = np.asarray(node_pos)
    buckets = (np.asarray(node_ids) // GRP).astype(np.int64)          # [T, P]
    in_maps = []
    for core in range(NCORES):
        xs = np.empty((TLOC, P, D), np.float16)
        bid = np.empty((TLOC, 128, NT), np.float32)
        cnt = np.empty((TLOC, L), np.float32)
        for i, t in enumerate(range(core * TLOC, (core + 1) * TLOC)):
            xs[i] = x[t][node_pos[t]]
            bid[i] = buckets[t].astype(np.float32).reshape(128, NT)
            cnt[i] = np.bincount(buckets[t], minlength=L).astype(np.float32)
        in_maps.append({
            "xs": xs,
            "bid": bid,
            "cnt": cnt.reshape(TLOC, L, 1),
            **consts,
        })
    return in_maps


def kernel(**inputs):
    in_maps = _host_prep(
        inputs["x"], inputs["g_enc"], inputs["b_enc"], inputs["g_fin"], inputs["b_fin"],
        inputs["g_dec"], inputs["b_dec"], inputs["node_pos"], inputs["node_ids"])
    nc = _get_nc()
    res = run_bass_kernel_spmd(nc, in_maps, core_ids=list(range(NCORES)))
    out = np.concatenate([r["y"] for r in res.results], axis=0)
    return out.astype(np.float32)
